# revision 16
# baseline (speedup 1.0000x reference)
"""BinAlexNet Trainium2 kernel — 8-core data-parallel SPMD (Bass/Tile).

Strategy (v2):
- Data-parallel over batch (64 images/core); per-layer BN batch stats via
  small AllGathers of per-core partial sums.
- Layer 1 (fp32) reproduces the XLA lowering bit-exactly: K=75 fp32 matmul
  with the XLA im2col K order and reduction structure (unchanged from v1).
- Layers 2-5 use a position-major activation layout [ch, i, j, img] so that
  conv tap deltas are multiples of 64 elements.  That satisfies the HW
  DoubleRow constraint (3D ifmap AP [K, 2, N], pair step % 16 == 0), letting
  each fp8 matmul contract TWO taps at 0.5 cycles/row: 5x5 conv = 12 DR + 1
  normal matmul, 3x3 conv = 4 DR + 1 normal, all with free dim 384.
- All later-layer arithmetic is exact integer (signs in {-1,0,1} as fp8
  operands, fp32 PSUM), so results stay bit-exact.
- Engine spreading: PSUM->SBUF copies + all sign() on the Activation engine
  (sign(x - mean) as one ACT op with per-partition bias = -mean, exact),
  pooling maxes on DVE (one PSUM input max per instruction), memsets on
  GPSIMD.  Stats sums ride ACT accum_out where possible.
"""

import sys
import numpy as np

sys.path.insert(0, "/opt/trn_rl_repo")

import concourse.bass as bass
import concourse.mybir as mybir
import concourse.tile as tile
from concourse import bacc
from concourse import bass_utils

N_CORES = 8
B = 512
BC = B // N_CORES  # 64 images per core
F32 = mybir.dt.float32
F8 = mybir.dt.float8e4
NPF8 = np.dtype(mybir.dt.np(F8))

AX = mybir.AxisListType.X
OP = mybir.AluOpType
DR = mybir.MatmulPerfMode.DoubleRow
ACT_COPY = mybir.ActivationFunctionType.Copy

R115200 = float(np.float32(1.0 / 115200.0))
R18432 = float(np.float32(1.0 / 18432.0))
R2048 = float(np.float32(1.0 / 2048.0))
R512 = float(np.float32(1.0 / 512.0))

_CACHE = {}


def _custom_ap(base_ap, offset, dims):
    vec = type(base_ap.ap)(dims)
    return bass.AP(tensor=base_ap.tensor, offset=offset, ap=vec)


def _pview(t, elem_base, dims):
    """Custom AP over SBUF tile t at element offset: dims = [[step,count]..]
    appended after the partition dim."""
    full = t[:]
    vec = type(full.ap)([[full.ap[0][0], full.ap[0][1]]] + dims)
    return bass.AP(tensor=full.tensor, offset=full.offset + elem_base, ap=vec)


# 5x5 tap pairs (kh*5+kw indices): vertical pairs rows 0-1 and 2-3, then
# horizontal pairs in row 4, plus single tap 24.
PAIRS5 = [(t, t + 5) for t in range(5)] + \
         [(t, t + 5) for t in range(10, 15)] + [(20, 21), (22, 23)]
# 3x3 tap pairs (kh*3+kw): vertical rows 0-1, horizontal pair in row 2.
PAIRS3 = [(0, 3), (1, 4), (2, 5), (6, 7)]


def _stats_ag(nc, statp, dramp, rs, recips, name):
    """AllGather per-core partial sums; rs = [( [P,1] sum tile, P)]; returns
    per-block NEGATED means [P,1] (bias for ACT sign)."""
    C = sum(P for _, P in rs)
    ag_in = dramp.tile([1, C], F32, tag=f"{name}_agi")
    ag_out = dramp.tile([N_CORES, C], F32, tag=f"{name}_ago",
                        addr_space="Shared")
    off = 0
    for r, P in rs:
        nc.sync.dma_start(ag_in[:, off:off + P].rearrange("one c -> c one"),
                          r[:])
        off += P
    nc.gpsimd.collective_compute(
        "AllGather", OP.bypass, replica_groups=[list(range(N_CORES))],
        ins=[ag_in.opt()], outs=[ag_out.opt()])
    ms = []
    off = 0
    for i, (r, P) in enumerate(rs):
        t_ = statp.tile([P, N_CORES], F32, tag=f"{name}_t{i}")
        nc.sync.dma_start(t_[:],
                          ag_out[:, off:off + P].rearrange("r c -> c r"))
        m_ = statp.tile([P, 1], F32, tag=f"{name}_m{i}")
        nc.vector.reduce_sum(m_[:], t_[:], axis=AX)
        nc.vector.tensor_scalar_mul(m_[:], m_[:], -recips[i])
        ms.append(m_)
        off += P
    return ms


def _conv3_pos(nc, tc, statp, dramp, src_blocks, w_t, cout, name,
               pool3, out_blocks, recip):
    """3x3 pad-1 binary conv on position-major 8x8 planes.
    src_blocks: [(tile [P, 8, 8, BC] fp8, P)].
    pool3=False: out_blocks = [[P, 8, 8, BC] fp8 padded planes], sign applied.
    pool3=True:  out_blocks = [[P, 4, BC] fp8 pos-major pooled], sign applied.
    """
    n_mb = (cout + 127) // 128
    nrows = 5 if pool3 else 6
    with tc.tile_pool(name=f"{name}w", bufs=1) as wpool, \
         tc.tile_pool(name=f"{name}h", bufs=1) as hp, \
         tc.tile_pool(name=f"{name}t", bufs=2) as tp, \
         tc.tile_pool(name=f"{name}ps", bufs=1, space="PSUM") as pp:
        w_sb = []
        for i, (_, P) in enumerate(src_blocks):
            off = sum(p for _, p in src_blocks[:i])
            w = wpool.tile([P, 9, cout], F8, tag=f"{name}_w{i}")
            nc.sync.dma_start(
                w[:], w_t.ap()[:, off:off + P].rearrange("t c o -> c t o"))
            w_sb.append(w)
        n_slots = 5 * len(src_blocks)
        hs, accs = [], []
        for mb in range(n_mb):
            MP = min(128, cout - 128 * mb)
            mlo = mb * 128
            if pool3:
                h = hp.tile([MP, 2, 2, BC], F32, tag=f"{name}_h{mb}",
                            name=f"{name}_h{mb}")
                acc = statp.tile([MP, 1], F32, tag=f"{name}_a{mb}")
            else:
                h = hp.tile([MP, 6, 6, BC], F32, tag=f"{name}_h{mb}",
                            name=f"{name}_h{mb}")
                acc = statp.tile([MP, nrows], F32, tag=f"{name}_a{mb}")
            hs.append(h)
            accs.append(acc)
            pstiles = []
            for i in range(nrows):
                ps = pp.tile([MP, 6 * BC], F32, tag=f"{name}ps{i}",
                             name=f"{name}ps{i}")
                k = 0
                for bi, (s_, P) in enumerate(src_blocks):
                    for (ta, tb) in PAIRS3:
                        kha, kwa = divmod(ta, 3)
                        khb, kwb = divmod(tb, 3)
                        base = ((i + kha) * 8 + kwa) * BC
                        d = ((khb - kha) * 8 + (kwb - kwa)) * BC
                        rhs = _pview(s_, base, [[d, 2], [1, 6 * BC]])
                        st = tb - ta
                        lhs = w_sb[bi][:, ta:tb + 1:st, mlo:mlo + MP]
                        nc.tensor.matmul(ps[:], lhs, rhs, start=(k == 0),
                                         stop=False, perf_mode=DR)
                        k += 1
                    base = ((i + 2) * 8 + 2) * BC
                    rhs = _pview(s_, base, [[1, 6 * BC]])
                    nc.tensor.matmul(ps[:], w_sb[bi][:, 8, mlo:mlo + MP],
                                     rhs, start=False,
                                     stop=(k == n_slots - 1))
                    k += 1
                pstiles.append(ps)
                if not pool3:
                    # copy out row + accumulate stats (ACT)
                    nc.scalar.activation(
                        h[:, i, :, :].rearrange("p j b -> p (j b)"),
                        ps[:], ACT_COPY, accum_out=acc[:, i:i + 1])
            if pool3:
                # 3x3/s2 pool on 6x6 -> 2x2 (rows 0-4 computed)
                rm = tp.tile([MP, 2, 6, BC], F32, tag=f"{name}_rm{mb}",
                             name=f"{name}_rm{mb}")
                ce = tp.tile([MP, 2, 6 * BC], F32, tag=f"{name}_ce{mb}",
                             name=f"{name}_ce{mb}")
                for g, (ra, rb) in enumerate(((0, 1), (3, 4))):
                    rmf = rm[:, g].rearrange("p j b -> p (j b)")
                    nc.scalar.copy(ce[:, g], pstiles[ra][:])
                    nc.vector.tensor_tensor(rmf, ce[:, g],
                                            pstiles[rb][:], OP.max)
                    nc.vector.tensor_tensor(rmf, rmf, pstiles[2][:],
                                            OP.max)
                u = tp.tile([MP, 2, 2, BC], F32, tag=f"{name}_u{mb}",
                            name=f"{name}_u{mb}")
                nc.vector.tensor_tensor(u[:], rm[:, :, 0:4:2, :],
                                        rm[:, :, 1:5:2, :], OP.max)
                nc.vector.tensor_tensor(h[:], u[:], rm[:, :, 2:5:2, :],
                                        OP.max)
                nc.vector.reduce_sum(
                    acc[:], h.rearrange("p r j b -> p (r j b)"), axis=AX)
        rsum = []
        for mb in range(n_mb):
            MP = hs[mb].shape[0]
            if pool3:
                rsum.append((accs[mb], MP))
            else:
                r = statp.tile([MP, 1], F32, tag=f"{name}_r{mb}")
                nc.vector.reduce_sum(r[:], accs[mb][:], axis=AX)
                rsum.append((r, MP))
        negms = _stats_ag(nc, statp, dramp, rsum,
                          [recip] * n_mb, name)
        for mb in range(n_mb):
            MP = hs[mb].shape[0]
            if pool3:
                nc.scalar.sign(
                    out_blocks[mb][:MP, :, :],
                    hs[mb].rearrange("p r j b -> p (r j) b"),
                    bias=negms[mb][:])
            else:
                nc.scalar.sign(out_blocks[mb][:MP, 1:7, 1:7, :], hs[mb][:],
                               bias=negms[mb][:])


def _border_memset(nc, t):
    """Zero only the padding border of a [P, H, W, BC] plane (GPSIMD)."""
    H = t.shape[1]
    nc.gpsimd.memset(t[:, 0:H:H - 1, :, :], 0.0)
    nc.gpsimd.memset(t[:, 1:H - 1, 0:H:H - 1, :], 0.0)


def _build(debug=False):
    nc = bacc.Bacc("TRN2", target_bir_lowering=False, debug=False,
                   num_devices=N_CORES)

    xp_t = nc.dram_tensor("xp", [65, 3, 34, 34], F32, kind="ExternalInput")
    w1_t = nc.dram_tensor("w1t", [75, 64], F32, kind="ExternalInput")
    w2_t = nc.dram_tensor("w2t", [25, 64, 192], F8, kind="ExternalInput")
    w3_t = nc.dram_tensor("w3t", [9, 192, 384], F8, kind="ExternalInput")
    w4_t = nc.dram_tensor("w4t", [9, 384, 256], F8, kind="ExternalInput")
    w5_t = nc.dram_tensor("w5t", [9, 256, 256], F8, kind="ExternalInput")
    wl1_t = nc.dram_tensor("wl1t", [4, 256, 4096], F8, kind="ExternalInput")
    wl2_t = nc.dram_tensor("wl2t", [4096, 2048], F8, kind="ExternalInput")
    wl3_t = nc.dram_tensor("wl3t", [2048, 10], F8, kind="ExternalInput")
    bl3_t = nc.dram_tensor("bl3c", [10, 1], F32, kind="ExternalInput")
    out_t = nc.dram_tensor("out", [BC, 10], F32, kind="ExternalOutput")

    with tile.TileContext(nc) as tc:
        import contextlib
        with contextlib.ExitStack() as stack:
            acts = stack.enter_context(tc.tile_pool(name="acts", bufs=1))
            const = stack.enter_context(tc.tile_pool(name="const", bufs=1))
            dramp = stack.enter_context(
                tc.tile_pool(name="dram", bufs=1, space="DRAM"))
            statp = stack.enter_context(tc.tile_pool(name="stat", bufs=1))

            ones = const.tile([128, 1], F32, tag="ones")
            nc.vector.memset(ones[:], 1.0)

            # ============ Layer 1 (fp32, bit-exact, image-major) ============
            s1p = acts.tile([64, 17, 17, BC], F8, tag="s1p")
            _border_memset(nc, s1p)
            with tc.tile_pool(name="l1ic", bufs=2) as icp, \
                 tc.tile_pool(name="l1h", bufs=2) as h1p, \
                 tc.tile_pool(name="l1p", bufs=1) as l1p, \
                 tc.tile_pool(name="ps1", bufs=4, space="PSUM") as pp1:
                w1_sb = const.tile([75, 64], F32, tag="w1")
                nc.sync.dma_start(w1_sb[:], w1_t.ap())
                p1 = l1p.tile([64, BC * 225], F32, tag="p1")
                r1 = statp.tile([64, BC, 1], F32, tag="r1")
                xp_flat = xp_t.ap().rearrange("n c r l -> (n c r l)")
                IT = 4  # images per tile
                for t in range(BC // IT):
                    ic = icp.tile([75, IT * 1020], F32, tag="ic")
                    for c in range(3):
                        for kh in range(5):
                            src = _custom_ap(
                                xp_flat,
                                t * IT * 3468 + c * 1156 + kh * 34,
                                [[1, 5], [3468, IT], [1, 1020]])
                            nc.sync.dma_start(
                                ic[c * 25 + kh * 5:c * 25 + kh * 5 + 5, :],
                                src)
                    h1 = h1p.tile([64, IT * 1020], F32, tag="h1")
                    for q in range(IT * 2):
                        ps = pp1.tile([64, 512], F32, tag="ps1")
                        nc.tensor.matmul(ps[:, :510], w1_sb[:],
                                         ic[:, q * 510:(q + 1) * 510],
                                         start=True, stop=True)
                        if q % 2 == 0:
                            nc.vector.tensor_copy(
                                h1[:, q * 510:(q + 1) * 510], ps[:, :510])
                        else:
                            nc.scalar.copy(
                                h1[:, q * 510:(q + 1) * 510], ps[:, :510])
                    hv = h1.rearrange("c (b r l) -> c b r l", r=30, l=34)
                    a00 = hv[:, :, 0:30:2, 0:30:2]
                    a01 = hv[:, :, 0:30:2, 1:30:2]
                    a10 = hv[:, :, 1:30:2, 0:30:2]
                    a11 = hv[:, :, 1:30:2, 1:30:2]
                    pv = p1.rearrange("c (b s) -> c b s", s=225)
                    pv = pv[:, t * IT:(t + 1) * IT].rearrange(
                        "c b (i j) -> c b i j", i=15, j=15)
                    tmp = h1p.tile([64, IT, 15, 15], F32, tag="pooltmp")
                    nc.vector.tensor_tensor(tmp[:], a00, a01, OP.max)
                    nc.vector.tensor_tensor(pv, a10, a11, OP.max)
                    nc.vector.tensor_tensor(pv, pv, tmp[:], OP.max)
                    # bit-exact per-image partial sums, overlapped with PE
                    nc.vector.reduce_sum(
                        r1[:, t * IT:(t + 1) * IT, :],
                        p1.rearrange("c (b s) -> c b s",
                                     s=225)[:, t * IT:(t + 1) * IT],
                        axis=AX)

                # ---- bit-exact layer-1 stats (XLA reduce structure) ----
                r1f = r1[:, :, 0]
                tl = statp.tile([64, 16], F32, tag="tl1")
                nc.vector.tensor_add(tl[:], r1f[:, 0:64:4], r1f[:, 1:64:4])
                nc.vector.tensor_add(tl[:], tl[:], r1f[:, 2:64:4])
                nc.vector.tensor_add(tl[:], tl[:], r1f[:, 3:64:4])
                ag1_in = dramp.tile([16, 64], F32, tag="ag1i")
                ag1_out = dramp.tile([128, 64], F32, tag="ag1o",
                                     addr_space="Shared")
                nc.sync.dma_start(ag1_in.rearrange("p c -> c p"), tl[:])
                nc.gpsimd.collective_compute(
                    "AllGather", OP.bypass,
                    replica_groups=[list(range(N_CORES))],
                    ins=[ag1_in.opt()], outs=[ag1_out.opt()])
                t_all = statp.tile([128, 64], F32, tag="tall1")
                nc.sync.dma_start(t_all[:], ag1_out[:])
                s1ps = pp1.tile([1, 64], F32, tag="s1sum")
                nc.tensor.matmul(s1ps[:], ones[:], t_all[:],
                                 start=True, stop=True)
                s1s = statp.tile([1, 64], F32, tag="s1s")
                nc.vector.tensor_copy(s1s[:], s1ps[:])
                m1_dram = dramp.tile([1, 64], F32, tag="m1b")
                nc.sync.dma_start(m1_dram[:], s1s[:])
                m1c = statp.tile([64, 1], F32, tag="m1c")
                nc.sync.dma_start(m1c[:],
                                  m1_dram.rearrange("one c -> c one"))
                negm1 = statp.tile([64, 1], F32, tag="negm1")
                nc.vector.tensor_scalar_mul(negm1[:], m1c[:], -R115200)

                # sign -> position-major padded plane (ACT), row chunks so
                # conv2's first rows can start before all rows are signed
                p1r = p1.rearrange("c (b i j) -> c i j b", i=15, j=15)
                for (r0, r1_) in ((0, 4), (4, 8), (8, 12), (12, 15)):
                    nc.scalar.sign(
                        s1p[:, 1 + r0:1 + r1_, 1:16, :],
                        p1r[:, r0:r1_, :, :], bias=negm1[:])

            # ============ Layer 2: conv2 5x5 (position-major, DR) ==========
            s2pA = acts.tile([128, 8, 8, BC], F8, tag="s2pA")
            s2pB = acts.tile([64, 8, 8, BC], F8, tag="s2pB")
            _border_memset(nc, s2pA)
            _border_memset(nc, s2pB)
            with tc.tile_pool(name="l2w", bufs=1) as l2w, \
                 tc.tile_pool(name="l2", bufs=2) as l2p, \
                 tc.tile_pool(name="ps2", bufs=2, space="PSUM") as pp2:
                w2_sb = l2w.tile([64, 25, 192], F8, tag="w2")
                nc.sync.dma_start(w2_sb[:],
                                  w2_t.ap().rearrange("t c o -> c t o"))
                h2a = l2p.tile([128, 6, 6, BC], F32, tag="h2a")
                h2b = l2p.tile([64, 6, 6, BC], F32, tag="h2b")
                N2 = 6 * BC
                for r in range(6):
                    for s in range(2):
                        js = 6 * s
                        pse = {}
                        for par in range(2):  # row 2r+par
                            i = 2 * r + par
                            pa = pp2.tile([128, N2], F32, tag=f"pa{par}",
                                          name=f"pa{par}")
                            pb = pp2.tile([64, N2], F32, tag=f"pb{par}",
                                          name=f"pb{par}")
                            k = 0
                            for (ta, tb) in PAIRS5:
                                kha, kwa = divmod(ta, 5)
                                khb, kwb = divmod(tb, 5)
                                base = ((i + kha) * 17 + kwa + js) * BC
                                d = ((khb - kha) * 17 + (kwb - kwa)) * BC
                                rhs = _pview(s1p, base, [[d, 2], [1, N2]])
                                st = tb - ta
                                nc.tensor.matmul(
                                    pa[:], w2_sb[:, ta:tb + 1:st, 0:128],
                                    rhs, start=(k == 0), stop=False,
                                    perf_mode=DR)
                                nc.tensor.matmul(
                                    pb[:], w2_sb[:, ta:tb + 1:st, 128:192],
                                    rhs, start=(k == 0), stop=False,
                                    perf_mode=DR)
                                k += 1
                            base = ((i + 4) * 17 + 4 + js) * BC
                            rhs = _pview(s1p, base, [[1, N2]])
                            nc.tensor.matmul(pa[:], w2_sb[:, 24, 0:128],
                                             rhs, start=False, stop=True)
                            nc.tensor.matmul(pb[:], w2_sb[:, 24, 128:192],
                                             rhs, start=False, stop=True)
                            pse[par] = (pa, pb)
                        for h2, half in ((h2a, 0), (h2b, 1)):
                            P = h2.shape[0]
                            ce = l2p.tile([P, N2], F32, tag=f"l2ce{P}",
                                          name=f"l2ce{P}")
                            rm = l2p.tile([P, N2], F32, tag=f"l2rm{P}",
                                          name=f"l2rm{P}")
                            nc.scalar.copy(ce[:], pse[0][half][:])
                            nc.vector.tensor_tensor(rm[:], ce[:],
                                                    pse[1][half][:], OP.max)
                            rv = rm.rearrange("p (j b) -> p j b", b=BC)
                            nc.vector.tensor_tensor(
                                h2[:, r, 3 * s:3 * s + 3, :],
                                rv[:, 0:6:2, :], rv[:, 1:6:2, :], OP.max)
                r2a = statp.tile([128, 1], F32, tag="r2a")
                r2b = statp.tile([64, 1], F32, tag="r2b")
                nc.vector.reduce_sum(
                    r2a[:], h2a.rearrange("p r j b -> p (r j b)"), axis=AX)
                nc.vector.reduce_sum(
                    r2b[:], h2b.rearrange("p r j b -> p (r j b)"), axis=AX)
                negm2 = _stats_ag(nc, statp, dramp,
                                  [(r2a, 128), (r2b, 64)],
                                  [R18432, R18432], "l2")
                nc.scalar.sign(s2pA[:, 1:7, 1:7, :], h2a[:],
                               bias=negm2[0][:])
                nc.scalar.sign(s2pB[:, 1:7, 1:7, :], h2b[:],
                               bias=negm2[1][:])

            # ============ Layers 3-5 (position-major, DR) ============
            # FC weight loads spread across conv3-5 so each conv layer's
            # own weight DMA is not queued behind 12MB of FC weights
            fcw = stack.enter_context(tc.tile_pool(name="fcw", bufs=1))
            wl1_sb = fcw.tile([128, 4, 2, 4096], F8, tag="wl1")
            nc.gpsimd.dma_start(
                wl1_sb[:],
                wl1_t.ap().rearrange("s (cb p) o -> p s cb o", p=128))
            wl2_sbs = []
            for half in range(2):
                w_ = fcw.tile([128, 32, 8 * 128], F8, tag=f"wl2_{half}",
                              name=f"wl2_{half}")
                wl2_sbs.append(w_)

            s3p = [acts.tile([128, 8, 8, BC], F8, tag=f"s3p{i}",
                             name=f"s3p{i}") for i in range(3)]
            for t in s3p:
                _border_memset(nc, t)
            _conv3_pos(nc, tc, statp, dramp, [(s2pA, 128), (s2pB, 64)],
                       w3_t, 384, "l3", False, s3p, R18432)

            nc.gpsimd.dma_start(
                wl2_sbs[0][:],
                wl2_t.ap()[:, 0:1024].rearrange("(kt p) m -> p kt m", p=128))
            s4p = [acts.tile([128, 8, 8, BC], F8, tag=f"s4p{i}",
                             name=f"s4p{i}") for i in range(2)]
            for t in s4p:
                _border_memset(nc, t)
            _conv3_pos(nc, tc, statp, dramp,
                       [(s3p[0], 128), (s3p[1], 128), (s3p[2], 128)],
                       w4_t, 256, "l4", False, s4p, R18432)

            nc.gpsimd.dma_start(
                wl2_sbs[1][:],
                wl2_t.ap()[:, 1024:2048].rearrange("(kt p) m -> p kt m",
                                                   p=128))
            s5 = [acts.tile([128, 4, BC], F8, tag=f"s5{i}",
                            name=f"s5{i}") for i in range(2)]
            _conv3_pos(nc, tc, statp, dramp,
                       [(s4p[0], 128), (s4p[1], 128)], w5_t, 256,
                       "l5", True, s5, R2048)

            # ============ FC block ============
            s6 = acts.tile([128, 32, BC], F8, tag="s6")
            with tc.tile_pool(name="fc1", bufs=2) as f1p, \
                 tc.tile_pool(name="psf1", bufs=8, space="PSUM") as ppf1:
                z1 = f1p.tile([128, 32, BC], F32, tag="z1")
                fa1 = statp.tile([128, 32], F32, tag="fa1")
                for mo in range(32):
                    ps = ppf1.tile([128, BC], F32, tag="psf1")
                    k = 0
                    for cb in range(2):
                        for s in range(4):
                            nc.tensor.matmul(
                                ps[:],
                                wl1_sb[:, s, cb, mo * 128:(mo + 1) * 128],
                                s5[cb][:, s, :], start=(k == 0),
                                stop=(k == 7))
                            k += 1
                    nc.scalar.activation(z1[:, mo, :], ps[:], ACT_COPY,
                                         accum_out=fa1[:, mo:mo + 1])
                negmz1 = _stats_fc(nc, statp, dramp, fa1, 32, R512, "fc1")
                for mo in range(32):
                    nc.scalar.sign(s6[:, mo, :], z1[:, mo, :],
                                   bias=negmz1[:, mo:mo + 1])

            s7 = acts.tile([128, 16, BC], F8, tag="s7")
            with tc.tile_pool(name="fc2", bufs=2) as f2p, \
                 tc.tile_pool(name="psf2", bufs=8, space="PSUM") as ppf2:
                z2 = f2p.tile([128, 16, BC], F32, tag="z2")
                fa2 = statp.tile([128, 16], F32, tag="fa2")
                for half in range(2):
                    wl2_sb = wl2_sbs[half]
                    for mo in range(8):
                        ps = ppf2.tile([128, BC], F32, tag="psf2")
                        for kt in range(32):
                            nc.tensor.matmul(
                                ps[:],
                                wl2_sb[:, kt, mo * 128:(mo + 1) * 128],
                                s6[:, kt, :],
                                start=(kt == 0), stop=(kt == 31))
                        ob = half * 8 + mo
                        nc.scalar.activation(z2[:, ob, :], ps[:], ACT_COPY,
                                             accum_out=fa2[:, ob:ob + 1])
                negmz2 = _stats_fc(nc, statp, dramp, fa2, 16, R512, "fc2")
                for ob in range(16):
                    nc.scalar.sign(s7[:, ob, :], z2[:, ob, :],
                                   bias=negmz2[:, ob:ob + 1])

            with tc.tile_pool(name="fc3", bufs=1) as f3p, \
                 tc.tile_pool(name="psf3", bufs=1, space="PSUM") as ppf3:
                wl3_sb = f3p.tile([128, 16, 10], F8, tag="wl3")
                nc.sync.dma_start(
                    wl3_sb[:],
                    wl3_t.ap().rearrange("(kt p) o -> p kt o", p=128))
                bl3_sb = f3p.tile([10, 1], F32, tag="bl3")
                nc.sync.dma_start(bl3_sb[:], bl3_t.ap())
                ps = ppf3.tile([10, BC], F32, tag="psf3")
                for kt in range(16):
                    nc.tensor.matmul(ps[:], wl3_sb[:, kt, :], s7[:, kt, :],
                                     start=(kt == 0), stop=(kt == 15))
                o_sb = f3p.tile([10, BC], F32, tag="osb")
                nc.vector.tensor_scalar(o_sb[:], ps[:], bl3_sb[:], None,
                                        op0=OP.add)
                nc.sync.dma_start(out_t.ap().rearrange("b o -> o b"),
                                  o_sb[:])

    nc.compile()
    return nc


def _stats_fc(nc, statp, dramp, r, nob, recip, name):
    """FC batch stats from per-core sums r [128, nob] -> negated means."""
    K = 128 * nob
    ag_in = dramp.tile([1, K], F32, tag=f"{name}_agi")
    ag_out = dramp.tile([N_CORES, K], F32, tag=f"{name}_ago",
                        addr_space="Shared")
    nc.sync.dma_start(ag_in.rearrange("one (ob p) -> p ob", p=128), r[:])
    nc.gpsimd.collective_compute(
        "AllGather", OP.bypass, replica_groups=[list(range(N_CORES))],
        ins=[ag_in.opt()], outs=[ag_out.opt()])
    t_ = statp.tile([128, N_CORES, nob], F32, tag=f"{name}_t")
    nc.sync.dma_start(t_[:], ag_out.rearrange("r (ob p) -> p r ob", p=128))
    m_ = statp.tile([128, nob], F32, tag=f"{name}_m")
    nc.vector.tensor_add(m_[:], t_[:, 0, :], t_[:, 1, :])
    for rr in range(2, N_CORES):
        nc.vector.tensor_add(m_[:], m_[:], t_[:, rr, :])
    nc.vector.tensor_scalar_mul(m_[:], m_[:], -recip)
    return m_


def _prep_inputs(inputs):
    f32 = np.float32
    x = np.asarray(inputs["x"], f32)
    sgn = np.sign
    w1s = sgn(np.asarray(inputs["w1"], f32)).astype(f32)
    w1t = np.ascontiguousarray(w1s.reshape(64, 75).T)
    w2t = np.ascontiguousarray(
        sgn(np.asarray(inputs["w2"], f32)).transpose(2, 3, 1, 0)
        .reshape(25, 64, 192)).astype(NPF8)
    w3t = np.ascontiguousarray(
        sgn(np.asarray(inputs["w3"], f32)).transpose(2, 3, 1, 0)
        .reshape(9, 192, 384)).astype(NPF8)
    w4t = np.ascontiguousarray(
        sgn(np.asarray(inputs["w4"], f32)).transpose(2, 3, 1, 0)
        .reshape(9, 384, 256)).astype(NPF8)
    w5t = np.ascontiguousarray(
        sgn(np.asarray(inputs["w5"], f32)).transpose(2, 3, 1, 0)
        .reshape(9, 256, 256)).astype(NPF8)
    wl1t = np.ascontiguousarray(
        sgn(np.asarray(inputs["wl1"], f32)).reshape(4096, 256, 4)
        .transpose(2, 1, 0)).astype(NPF8)
    wl2t = np.ascontiguousarray(
        sgn(np.asarray(inputs["wl2"], f32)).T).astype(NPF8)
    wl3t = np.ascontiguousarray(
        sgn(np.asarray(inputs["wl3"], f32)).T).astype(NPF8)
    bl3c = np.asarray(inputs["bl3"], f32).reshape(10, 1)
    shared = dict(w1t=w1t, w2t=w2t, w3t=w3t, w4t=w4t, w5t=w5t,
                  wl1t=wl1t, wl2t=wl2t, wl3t=wl3t, bl3c=bl3c)
    in_maps = []
    for c in range(N_CORES):
        xp = np.zeros((65, 3, 34, 34), f32)
        xp[:64, :, 1:33, 1:33] = x[c * BC:(c + 1) * BC]
        in_maps.append(dict(shared, xp=xp))
    return in_maps


def _get_nc(debug=False):
    key = ("nc", debug)
    if key not in _CACHE:
        _CACHE[key] = _build(debug=debug)
    return _CACHE[key]


class _Runner:
    """Persistent-jit SPMD runner (caches the jitted executable and
    device-resident inputs)."""

    def __init__(self, nc):
        import jax
        import concourse.mybir as mb
        from concourse import bass2jax
        from concourse.bass2jax import (_bass_exec_p, install_neuronx_cc_hook,
                                        partition_id_tensor)
        from jax.sharding import Mesh, PartitionSpec
        from jax.experimental.shard_map import shard_map
        install_neuronx_cc_hook()
        self.jax = jax
        self.nc = nc
        in_names, out_names, out_avals, zero_outs = [], [], [], []
        pname = nc.partition_id_tensor.name if nc.partition_id_tensor else None
        for alloc in nc.m.functions[0].allocations:
            if not isinstance(alloc, mb.MemoryLocationSet):
                continue
            name = alloc.memorylocations[0].name
            if alloc.kind == "ExternalInput":
                if name != pname:
                    in_names.append(name)
            elif alloc.kind == "ExternalOutput":
                shape = tuple(alloc.tensor_shape)
                dtype = mb.dt.np(alloc.dtype)
                out_names.append(name)
                out_avals.append(jax.core.ShapedArray(shape, dtype))
                zero_outs.append(np.zeros(shape, dtype))
        self.in_names, self.out_names = in_names, out_names
        self.out_avals, self.zero_outs = out_avals, zero_outs
        n_params, n_outs = len(in_names), len(out_avals)
        self.n_params = n_params
        all_names = list(in_names) + list(out_names)
        if pname is not None:
            all_names.append(pname)

        def _body(*args):
            operands = list(args)
            if pname is not None:
                operands.append(partition_id_tensor())
            outs = _bass_exec_p.bind(
                *operands, out_avals=tuple(out_avals),
                in_names=tuple(all_names), out_names=tuple(out_names),
                lowering_input_output_aliases=(),
                sim_require_finite=True, sim_require_nnan=True, nc=nc)
            return tuple(outs)

        devices = jax.devices()[:N_CORES]
        self.mesh = Mesh(np.asarray(devices), ("core",))
        in_specs = (PartitionSpec("core"),) * (n_params + n_outs)
        out_specs = (PartitionSpec("core"),) * n_outs
        donate = tuple(range(n_params, n_params + n_outs))
        self.fn = jax.jit(
            shard_map(_body, mesh=self.mesh, in_specs=in_specs,
                      out_specs=out_specs, check_rep=False),
            donate_argnums=donate, keep_unused=True)

    def put_inputs(self, in_maps):
        import jax
        from jax.sharding import NamedSharding, PartitionSpec
        sh = NamedSharding(self.mesh, PartitionSpec("core"))
        arrs = []
        for name in self.in_names:
            cat = np.concatenate([np.asarray(m[name]) for m in in_maps],
                                 axis=0)
            arrs.append(jax.device_put(cat, sh))
        return arrs

    def exec_once(self, dev_inputs):
        zeros = [np.zeros((N_CORES * z.shape[0], *z.shape[1:]), z.dtype)
                 for z in self.zero_outs]
        outs = self.fn(*dev_inputs, *zeros)
        return outs

    def run(self, in_maps):
        dev_inputs = self.put_inputs(in_maps)
        outs = self.exec_once(dev_inputs)
        results = [
            {name: np.asarray(outs[i]).reshape(
                N_CORES, *self.out_avals[i].shape)[c]
             for i, name in enumerate(self.out_names)}
            for c in range(N_CORES)]
        return results


def _get_runner(debug=False):
    key = ("runner", debug)
    if key not in _CACHE:
        _CACHE[key] = _Runner(_get_nc(debug=debug))
    return _CACHE[key]


class _Res:
    def __init__(self, results):
        self.results = results
        self.exec_time_ns = None
        self.profile_json = None
        self.instructions_and_trace = None


def run(inputs, debug=False, trace=False):
    runner = _get_runner(debug=debug)
    in_maps = _prep_inputs(inputs)
    results = runner.run(in_maps)
    res = _Res(results)
    out = np.concatenate([results[c]["out"] for c in range(N_CORES)], axis=0)
    return np.ascontiguousarray(out.astype(np.float32)), res


def time_exec(inputs, debug=False, iters=10):
    """Min wall time per execution with device-resident inputs."""
    import time as _t
    runner = _get_runner(debug=debug)
    in_maps = _prep_inputs(inputs)
    dev_inputs = runner.put_inputs(in_maps)
    outs = runner.exec_once(dev_inputs)  # warm/compile
    self_jax = runner.jax
    self_jax.block_until_ready(outs)
    best = float("inf")
    for _ in range(iters):
        t0 = _t.perf_counter()
        outs = runner.exec_once(dev_inputs)
        self_jax.block_until_ready(outs)
        best = min(best, _t.perf_counter() - t0)
    return best


def time_exec_sustained(inputs, debug=False, iters=200):
    """Sustained per-execution device time: pre-stage inputs and donated
    output buffers on device, issue `iters` executions back-to-back
    (async pipelined), block once, divide.  Amortizes host/transport
    dispatch overhead, leaving per-execution device time."""
    import time as _t
    import jax
    from jax.sharding import NamedSharding, PartitionSpec
    runner = _get_runner(debug=debug)
    in_maps = _prep_inputs(inputs)
    dev = runner.put_inputs(in_maps)
    sh = NamedSharding(runner.mesh, PartitionSpec("core"))
    zsets = [[jax.device_put(
        np.zeros((N_CORES * z.shape[0], *z.shape[1:]), z.dtype), sh)
        for z in runner.zero_outs] for _ in range(iters + 1)]
    outs = runner.fn(*dev, *zsets[-1])  # warm/compile
    jax.block_until_ready(outs)
    t0 = _t.perf_counter()
    all_outs = [runner.fn(*dev, *zsets[i]) for i in range(iters)]
    jax.block_until_ready(all_outs)
    return (_t.perf_counter() - t0) / iters


def kernel(**inputs):
    out, _ = run(inputs, debug=False)
    return out
